# revision 13
# baseline (speedup 1.0000x reference)
"""NUFFT multi-channel 3D layer on 8 Trainium2 NeuronCores.

Strategy: data-parallel over batch (8 batches -> 8 cores). Per core the whole
pipeline runs in the Fourier domain:
 - gaussians computed grid-major via one fused Derivative_Erf activation per
   periodic image (exp(-u^2) table), no transposes anywhere;
 - 1D DFT factor planes in both orientations obtained directly by matmuls
   (forward: F as stationary; transposed: g as stationary);
 - Khatri-Rao product h = az (x) ay in kz-major layout from broadcast views;
 - spread (particles->spectrum) and gather (spectrum->particles) as dense
   fp16 matmul streams, split into an hr pass and an hi pass so the tensor
   engine overlaps the DVE building hi; spectral multiply folded into fp16 W
   (hermitian weights on host, 1/N and sqrt(pi)/2 normalizations folded into
   the DFT matrices);
 - final per-particle contraction fused into one scalar_tensor_tensor with
   accum_out per (chunk, particle-half) on DVE.
Hermitian symmetry halves the kz axis (33 of 65 planes).
"""
import sys
import numpy as np

sys.path.insert(0, "/opt/trn_rl_repo")

N = 65
NH = 33
P = 256
B = 8
L = 2.0 * np.pi
TAU = float(np.float32(12.0 * (np.float32(L) / (2.0 * np.pi * N)) ** 2))
NCH = 2
KYZ = N * NH          # 2145
CH = 429              # free chunk (5 chunks)
NCHK = 5

_CACHE = {}


def _host_consts():
    j = np.arange(N, dtype=np.float64)
    m = np.arange(N, dtype=np.float64) - 32.0
    Lf = float(np.float32(L))
    ph = -2.0 * np.pi * np.outer(m, j) / N           # [k, j]
    # per-axis deconv; 1/N (fft normalization split) and sqrt(pi)/2
    # (Derivative_Erf = 2/sqrt(pi) exp(-u^2)) folded in.
    dec = (np.sqrt(np.pi / TAU) * np.exp(m * m * TAU)
           * (np.sqrt(np.pi) / 2.0) / N)
    Fr = np.cos(ph) * dec[:, None]                   # [k, j]
    Fi = np.sin(ph) * dec[:, None]
    # FF: Fxr | Fxi | Fzr | Fzi  => [65, 196] fp16
    FF = np.ascontiguousarray(
        np.concatenate([Fr.T, Fi.T, Fr.T[:, 32:], Fi.T[:, 32:]], axis=1),
        np.float16)
    xg = np.linspace(0.0, Lf, N + 1)[:-1].astype(np.float64)
    s2t = 1.0 / (2.0 * np.sqrt(TAU))
    shifts = np.array([0.0, Lf, -Lf])
    xb = (-(xg[:, None] + shifts[None, :]) * s2t)    # [65, 3]
    return dict(FF=FF, xb=xb)


def _make_w(Wfull):
    # device layout: [kx, (kz-half, ky)] with ky order 0:33=+ky, 33:65=-ky
    kyperm = list(range(32, 65)) + list(range(31, -1, -1))
    w = np.ones(NH); w[1:] = 2.0
    Wk = np.asarray(Wfull, np.float64)[:, kyperm, 32:] * w[None, None, :]
    Wk = Wk.transpose(0, 2, 1).reshape(N, KYZ)
    return np.ascontiguousarray(Wk.astype(np.float16))


def _trace_kernel():
    import concourse.bass as bass
    import concourse.bacc as bacc
    import concourse.tile as tile
    from concourse import mybir

    dt = mybir.dt
    f32 = dt.float32
    f16 = dt.float16
    AF = mybir.ActivationFunctionType
    OP = mybir.AluOpType
    AX = mybir.AxisListType

    nc = bacc.Bacc("TRN2", target_bir_lowering=False, debug=False)

    din = {}
    for name, shape, ddt in [
            ("ptsbx", (N, 771), f32),      # pts broadcast 768 | xb 3
            ("FF", (N, 196), f16),         # Fxr | Fxi | Fzr | Fzi
            ("W", (N, KYZ), f16)]:
        din[name] = nc.dram_tensor(name, list(shape), ddt,
                                   kind="ExternalInput").ap()
    dout = nc.dram_tensor("fmm", [P, 1], f32, kind="ExternalOutput").ap()

    s2t = float(1.0 / (2.0 * np.sqrt(TAU)))

    def v3(ap, b=33):
        return ap.rearrange("p (a b) -> p a b", b=b)

    with tile.TileContext(nc) as tc:
        with (
            tc.tile_pool(name="const", bufs=1) as cp,
            tc.tile_pool(name="glob", bufs=1) as gp,
            tc.tile_pool(name="eph", bufs=2) as ep,
            tc.tile_pool(name="hpl", bufs=2) as hp,
            tc.tile_pool(name="scr", bufs=1) as sp,
        ):
            # ---- constants (+ activation-table preload during DMA) ----
            ptsbx = cp.tile([N, 771], f32, tag="ptsbx")
            nc.sync.dma_start(ptsbx[:, 0:384], din["ptsbx"][:, 0:384])
            nc.scalar.dma_start(ptsbx[:, 384:771], din["ptsbx"][:, 384:771])
            dmy = sp.tile([128, 1], f32, tag="dmy")
            nc.vector.memset(dmy[:], 0.0)
            dmo = sp.tile([128, 1], f16, tag="dmo")
            nc.scalar.activation(dmo[:], dmy[:], AF.Derivative_Erf)
            nc.scalar.activation(dmo[:], dmy[:], AF.Copy, scale=-1.0)
            FF = cp.tile([N, 196], f16, tag="FF")
            nc.sync.dma_start(FF[:], din["FF"][:])
            Wt = cp.tile([N, KYZ], f16, tag="W")
            nc.sync.dma_start(Wt[:], din["W"][:])
            ptsb = ptsbx[:, 0:768]
            xb = ptsbx[:, 768:771]
            Fx, Fz = FF[:, 0:130], FF[:, 130:196]

            # ---- phase A: periodic gaussians, grid-major [x, (a p)] ----
            e3 = []
            for s in range(3):
                e = gp.tile([N, 768], f16, tag=f"e{s}")
                nc.scalar.activation(e[:], ptsb, AF.Derivative_Erf,
                                     bias=xb[:, s:s + 1], scale=s2t)
                e3.append(e)
            g = gp.tile([N, 768], f16, tag="g")
            nc.vector.tensor_tensor(g[:], e3[0][:], e3[1][:], op=OP.add)
            nc.vector.tensor_tensor(g[:], g[:], e3[2][:], op=OP.add)
            gx, gy, gz = g[:, 0:256], g[:, 256:512], g[:, 512:768]

            aT = []
            rep = []
            with tc.tile_pool(name="psC", bufs=1, space="PSUM") as psC:
                # ---- phase C: transposed DFT planes per particle chunk ----
                # aT[c] cols: axTr 0:65 | axTi 65:130 | ayTr 130:195 |
                #             ayTi 195:260 | azTr 260:293 | azTi 293:326 |
                #             naxTi 326:391
                for c in range(2):
                    cs = slice(c * 128, (c + 1) * 128)
                    pT = psC.tile([128, 326], f32, tag="pT", name=f"pT{c}")
                    nc.tensor.matmul(pT[:, 0:130], gx[:, cs], Fx,
                                     start=True, stop=True)
                    nc.tensor.matmul(pT[:, 130:260], gy[:, cs], Fx,
                                     start=True, stop=True)
                    nc.tensor.matmul(pT[:, 260:326], gz[:, cs], Fz,
                                     start=True, stop=True)
                    t = gp.tile([128, 391], f16, tag=f"aT{c}")
                    nc.scalar.copy(t[:, 0:326], pT[:])
                    nc.scalar.activation(t[:, 326:391], pT[:, 65:130],
                                         AF.Copy, scale=-1.0)
                    aT.append(t)
                    # az replicated along ky (scalar) so the DVE Khatri-Rao
                    # products run in the packed-fp16 2x mode
                    rr = ep.tile([128, 1089], f16, tag="rr", name=f"rr{c}")
                    nc.scalar.copy(
                        v3(rr[:]),
                        t[:, 260:293].unsqueeze(2).broadcast_to([128, 33, 33]))
                    ri = ep.tile([128, 1089], f16, tag="ri", name=f"ri{c}")
                    nc.scalar.copy(
                        v3(ri[:]),
                        t[:, 293:326].unsqueeze(2).broadcast_to([128, 33, 33]))
                    rep.append((rr, ri))
                # ---- forward ax planes [kx, p] ----
                psax = psC.tile([N, 512], f32, tag="psax", name="psax")
                nc.tensor.matmul(psax[:, 0:256], Fx[:, 0:65], gx,
                                 start=True, stop=True)
                nc.tensor.matmul(psax[:, 256:512], Fx[:, 65:130], gx,
                                 start=True, stop=True)
                # ax cols: axr 0:256 | axi 256:512 | naxi 512:768
                ax = gp.tile([N, 768], f16, tag="ax")
                nc.scalar.copy(ax[:, 0:512], psax[:])
                nc.scalar.activation(ax[:, 512:768], psax[:, 256:512],
                                     AF.Copy, scale=-1.0)

            # ---- phase E: h = az (x) ay, kz-major [p, (kz, ky65)] ----
            # DVE order: P1,P2 + hr combines per c first (unblocks the
            # spread's hr matmul pass), then P3,P4 + hi combines.
            hc, PP = [], []
            for c in range(2):
                t = aT[c]
                rr, ri = rep[c]
                ayr_b = t[:, 162:195].unsqueeze(1).broadcast_to([128, 33, 33])
                ayi_b = t[:, 227:260].unsqueeze(1).broadcast_to([128, 33, 33])
                P1 = ep.tile([128, 1089], f16, tag="P1", name=f"P1_{c}")
                P2 = ep.tile([128, 1089], f16, tag="P2", name=f"P2_{c}")
                nc.vector.tensor_tensor(v3(P1[:]), ayr_b, v3(rr[:]),
                                        op=OP.mult)
                nc.vector.tensor_tensor(v3(P2[:]), ayi_b, v3(ri[:]),
                                        op=OP.mult)
                hcat = hp.tile([128, 2 * KYZ], f16, tag="hcat",
                               name=f"hcat{c}")
                hrv = v3(hcat[:, 0:KYZ], b=N)     # [p, kz, ky]
                # +ky block: hr = P1 - P2 ; -ky block (ky 1:33): P1 + P2
                nc.vector.tensor_tensor(hrv[:, :, 0:33], v3(P1[:]),
                                        v3(P2[:]), op=OP.subtract)
                nc.vector.tensor_tensor(hrv[:, :, 33:65],
                                        v3(P1[:])[:, :, 1:33],
                                        v3(P2[:])[:, :, 1:33], op=OP.add)
                hc.append(hcat)
                PP.append((ayr_b, ayi_b))
            for c in range(2):
                ayr_b, ayi_b = PP[c]
                rr, ri = rep[c]
                P3 = ep.tile([128, 1089], f16, tag="P3", name=f"P3_{c}")
                P4 = ep.tile([128, 1089], f16, tag="P4", name=f"P4_{c}")
                nc.vector.tensor_tensor(v3(P3[:]), ayi_b, v3(rr[:]),
                                        op=OP.mult)
                nc.vector.tensor_tensor(v3(P4[:]), ayr_b, v3(ri[:]),
                                        op=OP.mult)
                hiv = v3(hc[c][:, KYZ:2 * KYZ], b=N)
                # +ky: hi = P4 + P3 ; -ky: hi = P4 - P3
                nc.vector.tensor_tensor(hiv[:, :, 0:33], v3(P4[:]),
                                        v3(P3[:]), op=OP.add)
                nc.vector.tensor_tensor(hiv[:, :, 33:65],
                                        v3(P4[:])[:, :, 1:33],
                                        v3(P3[:])[:, :, 1:33],
                                        op=OP.subtract)
            hr = [hc[c][:, 0:KYZ] for c in range(2)]
            hi = [hc[c][:, KYZ:2 * KYZ] for c in range(2)]

            with tc.tile_pool(name="psM", bufs=4, space="PSUM") as psM:
                # ---- phase F: spread + spectral multiply ----
                # hr pass over 4 chunks overlaps DVE building hi; psum ring
                # (4 x 2 banks) shared with the gather phase.
                Vc = gp.tile([N, 2 * KYZ], f16, tag="Vc")
                Vr, Vi = Vc[:, 0:KYZ], Vc[:, KYZ:2 * KYZ]
                pf = []

                def fmm_hr(k):
                    t = psM.tile([128, 1024], f32, tag="pq", name=f"pf{k}")
                    pf.append(t)
                    psr, psi = t[0:N, 0:CH], t[0:N, 512:512 + CH]
                    ch = slice(k * CH, (k + 1) * CH)
                    for c in range(2):
                        a = aT[c]
                        st = (c == 0)
                        nc.tensor.matmul(psr, a[:, 0:65], hr[c][:, ch],
                                         start=st, stop=False)
                        nc.tensor.matmul(psi, a[:, 65:130], hr[c][:, ch],
                                         start=st, stop=False)

                def fmm_hi(k):
                    t = pf[k]
                    psr, psi = t[0:N, 0:CH], t[0:N, 512:512 + CH]
                    ch = slice(k * CH, (k + 1) * CH)
                    for c in range(2):
                        a = aT[c]
                        sp_ = (c == 1)
                        nc.tensor.matmul(psr, a[:, 326:391], hi[c][:, ch],
                                         start=False, stop=sp_)
                        nc.tensor.matmul(psi, a[:, 0:65], hi[c][:, ch],
                                         start=False, stop=sp_)
                    # one DVE op: [Vr|Vi]_ch = [psr|psi] * W_ch
                    nc.vector.tensor_tensor(
                        v3(Vc[:], b=KYZ)[:, :, ch],
                        v3(t[0:N, :], b=512)[:, :, 0:CH],
                        Wt[:, ch].unsqueeze(1).broadcast_to([N, 2, CH]),
                        op=OP.mult)

                for k in range(4):
                    fmm_hr(k)
                for k in range(4):
                    fmm_hi(k)
                fmm_hr(4)
                fmm_hi(4)

                # ---- phase G + H: gather + fused multiply-reduce ----
                accT = []
                scr = []
                for c in range(2):
                    accT.append(sp.tile([128, NCHK], f32, tag=f"accT{c}",
                                        name=f"accT{c}"))
                    scr.append(sp.tile([128, 1024], f32, tag=f"scr{c}",
                                       name=f"scr{c}"))
                for k in range(NCHK):
                    ch = slice(k * CH, (k + 1) * CH)
                    for c in range(2):
                        axr_c = ax[:, c * 128:(c + 1) * 128]
                        axi_c = ax[:, 256 + c * 128:384 + c * 128]
                        naxi_c = ax[:, 512 + c * 128:640 + c * 128]
                        pg = psM.tile([128, 1024], f32, tag="pq",
                                      name=f"pg{c}_{k}")
                        pr, pi = pg[:, 0:CH], pg[:, 512:512 + CH]
                        nc.tensor.matmul(pr, axr_c, Vr[:, ch],
                                         start=True, stop=False)
                        nc.tensor.matmul(pr, axi_c, Vi[:, ch],
                                         start=False, stop=True)
                        nc.tensor.matmul(pi, axr_c, Vi[:, ch],
                                         start=True, stop=False)
                        nc.tensor.matmul(pi, naxi_c, Vr[:, ch],
                                         start=False, stop=True)
                        # one DVE op: accT[:,k] = sum(pr*hr + pi*hi)
                        nc.vector.scalar_tensor_tensor(
                            v3(scr[c][:], b=512)[:, :, 0:CH],
                            v3(pg[:], b=512)[:, :, 0:CH], 1.0,
                            v3(hc[c][:], b=KYZ)[:, :, ch],
                            op0=OP.mult, op1=OP.mult,
                            accum_out=accT[c][:, k:k + 1])
                for c in range(2):
                    fmm_c = sp.tile([128, 1], f32, tag=f"fmm{c}",
                                    name=f"fmm_{c}")
                    nc.vector.reduce_sum(fmm_c[:], accT[c][:], axis=AX.X)
                    nc.sync.dma_start(dout[c * 128:(c + 1) * 128, :],
                                      fmm_c[:])

    nc.compile()
    return nc


def _get_nc():
    if "nc" not in _CACHE:
        _CACHE["nc"] = _trace_kernel()
    return _CACHE["nc"]


def kernel(points, multRe0, multIm0, multRe1, multIm1):
    from concourse.bass_utils import run_bass_kernel_spmd

    points = np.asarray(points)
    multRe0 = np.asarray(multRe0)
    multRe1 = np.asarray(multRe1)
    multIm0 = np.asarray(multIm0)
    multIm1 = np.asarray(multIm1)

    Wfull = multRe0[0]
    ok = (np.all(multIm0 == 0) and np.all(multIm1 == 0)
          and np.array_equal(multRe0, multRe1)
          and np.array_equal(Wfull, Wfull[::-1, ::-1, ::-1]))
    if not ok:
        raise NotImplementedError("kernel specialized to symmetric real "
                                  "multipliers with equal channels")

    consts = _host_consts()
    Wk = _make_w(Wfull)

    ptsbx = np.zeros((B, N, 771), np.float32)
    for b in range(B):
        ptsbx[b, :, 0:768] = points[b].T.reshape(1, 768)
        ptsbx[b, :, 768:771] = consts["xb"]

    in_maps = []
    for b in range(B):
        in_maps.append({"ptsbx": ptsbx[b], "FF": consts["FF"], "W": Wk})

    nc = _get_nc()
    res = run_bass_kernel_spmd(nc, in_maps, core_ids=list(range(B)),
                               **_CACHE.get("run_kwargs", {}))
    _CACHE["last_result"] = res
    out = np.zeros((B, P, NCH), np.float32)
    for b in range(B):
        f = res.results[b]["fmm"][:, 0]
        out[b, :, 0] = f
        out[b, :, 1] = f
    return out


# revision 14
# speedup vs baseline: 1.1319x; 1.1319x over previous
"""NUFFT multi-channel 3D layer on 8 Trainium2 NeuronCores.

Strategy: data-parallel over batch (8 batches -> 8 cores). Per core the whole
pipeline runs in the Fourier domain:
 - gaussians computed grid-major via one fused Derivative_Erf activation per
   periodic image (exp(-u^2) table), no transposes anywhere;
 - 1D DFT factor planes in both orientations obtained directly by matmuls
   (forward: F as stationary; transposed: g as stationary);
 - Khatri-Rao product h = az (x) ay in kz-major layout; az replicated along
   ky by the scalar engine so the DVE products hit the packed-fp16 2x mode;
 - spread (particles->spectrum) and gather (spectrum->particles) as dense
   fp16 matmul streams, split into an hr pass and an hi pass so the tensor
   engine overlaps the DVE building hi; spectral multiply folded into fp16 W
   (hermitian weights on host, 1/N and sqrt(pi)/2 normalizations folded into
   the DFT matrices);
 - spread/gather PSUM pairs packed [re | im] into one 2-bank tile so the
   spectral multiply and the final contraction are one DVE op per chunk
   (scalar_tensor_tensor with accum_out for the fused multiply-reduce);
 - one shared 4-deep PSUM ring (8 banks) pipelines both matmul phases.
Hermitian symmetry halves the kz axis (33 of 65 planes).
"""
import sys
import numpy as np

sys.path.insert(0, "/opt/trn_rl_repo")

N = 65
NH = 33
P = 256
B = 8
L = 2.0 * np.pi
TAU = float(np.float32(12.0 * (np.float32(L) / (2.0 * np.pi * N)) ** 2))
NCH = 2
KYZ = N * NH          # 2145
CH = 429              # free chunk (5 chunks)
NCHK = 5

_CACHE = {}


def _host_consts():
    j = np.arange(N, dtype=np.float64)
    m = np.arange(N, dtype=np.float64) - 32.0
    Lf = float(np.float32(L))
    ph = -2.0 * np.pi * np.outer(m, j) / N           # [k, j]
    # per-axis deconv; 1/N (fft normalization split) and sqrt(pi)/2
    # (Derivative_Erf = 2/sqrt(pi) exp(-u^2)) folded in.
    dec = (np.sqrt(np.pi / TAU) * np.exp(m * m * TAU)
           * (np.sqrt(np.pi) / 2.0) / N)
    Fr = np.cos(ph) * dec[:, None]                   # [k, j]
    Fi = np.sin(ph) * dec[:, None]
    # FF: Fxr | Fxi | Fzr | Fzi  => [65, 196] fp16
    FF = np.ascontiguousarray(
        np.concatenate([Fr.T, Fi.T, Fr.T[:, 32:], Fi.T[:, 32:]], axis=1),
        np.float16)
    xg = np.linspace(0.0, Lf, N + 1)[:-1].astype(np.float64)
    s2t = 1.0 / (2.0 * np.sqrt(TAU))
    shifts = np.array([0.0, Lf, -Lf])
    xb = (-(xg[:, None] + shifts[None, :]) * s2t)    # [65, 3]
    return dict(FF=FF, xb=xb)


def _make_w(Wfull):
    # device layout: [kx, (kz-half, ky)] with ky order 0:33=+ky, 33:65=-ky
    kyperm = list(range(32, 65)) + list(range(31, -1, -1))
    w = np.ones(NH); w[1:] = 2.0
    Wk = np.asarray(Wfull, np.float64)[:, kyperm, 32:] * w[None, None, :]
    Wk = Wk.transpose(0, 2, 1).reshape(N, KYZ)
    return np.ascontiguousarray(Wk.astype(np.float16))


def _trace_kernel():
    import concourse.bass as bass
    import concourse.bacc as bacc
    import concourse.tile as tile
    from concourse import mybir

    dt = mybir.dt
    f32 = dt.float32
    f16 = dt.float16
    AF = mybir.ActivationFunctionType
    OP = mybir.AluOpType
    AX = mybir.AxisListType

    nc = bacc.Bacc("TRN2", target_bir_lowering=False, debug=False)

    din = {}
    for name, shape, ddt in [
            ("ptsbx", (N, 771), f32),      # pts broadcast 768 | xb 3
            ("FF", (N, 196), f16),         # Fxr | Fxi | Fzr | Fzi
            ("W", (N, KYZ), f16)]:
        din[name] = nc.dram_tensor(name, list(shape), ddt,
                                   kind="ExternalInput").ap()
    dout = nc.dram_tensor("fmm", [P, 1], f32, kind="ExternalOutput").ap()

    s2t = float(1.0 / (2.0 * np.sqrt(TAU)))

    def v3(ap, b=33):
        return ap.rearrange("p (a b) -> p a b", b=b)

    with tile.TileContext(nc) as tc:
        with (
            tc.tile_pool(name="const", bufs=1) as cp,
            tc.tile_pool(name="glob", bufs=1) as gp,
            tc.tile_pool(name="eph", bufs=2) as ep,
            tc.tile_pool(name="hpl", bufs=2) as hp,
            tc.tile_pool(name="scr", bufs=1) as sp,
        ):
            # ---- constants (+ activation-table preload during DMA) ----
            ptsbx = cp.tile([N, 771], f32, tag="ptsbx")
            nc.sync.dma_start(ptsbx[:, 0:384], din["ptsbx"][:, 0:384])
            nc.scalar.dma_start(ptsbx[:, 384:771], din["ptsbx"][:, 384:771])
            dmy = sp.tile([128, 1], f32, tag="dmy")
            nc.vector.memset(dmy[:], 0.0)
            dmo = sp.tile([128, 1], f16, tag="dmo")
            nc.scalar.activation(dmo[:], dmy[:], AF.Derivative_Erf)
            nc.scalar.activation(dmo[:], dmy[:], AF.Copy, scale=-1.0)
            FF = cp.tile([N, 196], f16, tag="FF")
            nc.sync.dma_start(FF[:], din["FF"][:])
            Wt = cp.tile([N, KYZ], f16, tag="W")
            nc.sync.dma_start(Wt[:], din["W"][:])
            ptsb = ptsbx[:, 0:768]
            xb = ptsbx[:, 768:771]
            Fx, Fz = FF[:, 0:130], FF[:, 130:196]

            # ---- phase A: periodic gaussians, grid-major [x, (a p)] ----
            e3 = []
            for s in range(3):
                e = gp.tile([N, 768], f16, tag=f"e{s}")
                nc.scalar.activation(e[:], ptsb, AF.Derivative_Erf,
                                     bias=xb[:, s:s + 1], scale=s2t)
                e3.append(e)
            g = gp.tile([N, 768], f16, tag="g")
            nc.vector.tensor_tensor(g[:], e3[0][:], e3[1][:], op=OP.add)
            nc.vector.tensor_tensor(g[:], g[:], e3[2][:], op=OP.add)
            gx, gy, gz = g[:, 0:256], g[:, 256:512], g[:, 512:768]

            aT = []
            rep = []
            with tc.tile_pool(name="psC", bufs=1, space="PSUM") as psC:
                # ---- phase C: transposed DFT planes per particle chunk ----
                # aT[c] cols: axTr 0:65 | axTi 65:130 | ayTr 130:195 |
                #             ayTi 195:260 | azTr 260:293 | azTi 293:326 |
                #             naxTi 326:391
                for c in range(2):
                    cs = slice(c * 128, (c + 1) * 128)
                    pT = psC.tile([128, 326], f32, tag="pT", name=f"pT{c}")
                    nc.tensor.matmul(pT[:, 0:130], gx[:, cs], Fx,
                                     start=True, stop=True)
                    nc.tensor.matmul(pT[:, 130:260], gy[:, cs], Fx,
                                     start=True, stop=True)
                    nc.tensor.matmul(pT[:, 260:326], gz[:, cs], Fz,
                                     start=True, stop=True)
                    t = gp.tile([128, 391], f16, tag=f"aT{c}")
                    nc.scalar.copy(t[:, 0:326], pT[:])
                    nc.scalar.activation(t[:, 326:391], pT[:, 65:130],
                                         AF.Copy, scale=-1.0)
                    aT.append(t)
                    # az replicated along ky (scalar) so the DVE Khatri-Rao
                    # products run in the packed-fp16 2x mode
                    rr = ep.tile([128, 1089], f16, tag="rr", name=f"rr{c}")
                    nc.scalar.copy(
                        v3(rr[:]),
                        t[:, 260:293].unsqueeze(2).broadcast_to([128, 33, 33]))
                    ri = ep.tile([128, 1089], f16, tag="ri", name=f"ri{c}")
                    nc.scalar.copy(
                        v3(ri[:]),
                        t[:, 293:326].unsqueeze(2).broadcast_to([128, 33, 33]))
                    rep.append((rr, ri))
                # ---- forward ax planes [kx, p] ----
                psax = psC.tile([N, 512], f32, tag="psax", name="psax")
                nc.tensor.matmul(psax[:, 0:256], Fx[:, 0:65], gx,
                                 start=True, stop=True)
                nc.tensor.matmul(psax[:, 256:512], Fx[:, 65:130], gx,
                                 start=True, stop=True)
                # ax cols: axr 0:256 | axi 256:512 | naxi 512:768
                ax = gp.tile([N, 768], f16, tag="ax")
                nc.scalar.copy(ax[:, 0:512], psax[:])
                nc.scalar.activation(ax[:, 512:768], psax[:, 256:512],
                                     AF.Copy, scale=-1.0)

            # ---- phase E: h = az (x) ay, kz-major [p, (kz, ky65)] ----
            # DVE order: P1,P2 + hr combines per c first (unblocks the
            # spread's hr matmul pass), then P3,P4 + hi combines.
            hc, PP = [], []
            for c in range(2):
                t = aT[c]
                rr, ri = rep[c]
                ayr_b = t[:, 162:195].unsqueeze(1).broadcast_to([128, 33, 33])
                ayi_b = t[:, 227:260].unsqueeze(1).broadcast_to([128, 33, 33])
                P1 = ep.tile([128, 1089], f16, tag="P1", name=f"P1_{c}")
                P2 = ep.tile([128, 1089], f16, tag="P2", name=f"P2_{c}")
                nc.vector.tensor_tensor(v3(P1[:]), ayr_b, v3(rr[:]),
                                        op=OP.mult)
                nc.vector.tensor_tensor(v3(P2[:]), ayi_b, v3(ri[:]),
                                        op=OP.mult)
                hcat = hp.tile([128, 2 * KYZ], f16, tag="hcat",
                               name=f"hcat{c}")
                hrv = v3(hcat[:, 0:KYZ], b=N)     # [p, kz, ky]
                # +ky block: hr = P1 - P2 ; -ky block (ky 1:33): P1 + P2
                nc.vector.tensor_tensor(hrv[:, :, 0:33], v3(P1[:]),
                                        v3(P2[:]), op=OP.subtract)
                nc.vector.tensor_tensor(hrv[:, :, 33:65],
                                        v3(P1[:])[:, :, 1:33],
                                        v3(P2[:])[:, :, 1:33], op=OP.add)
                hc.append(hcat)
                PP.append((ayr_b, ayi_b))
            for c in range(2):
                ayr_b, ayi_b = PP[c]
                rr, ri = rep[c]
                P3 = ep.tile([128, 1089], f16, tag="P3", name=f"P3_{c}")
                P4 = ep.tile([128, 1089], f16, tag="P4", name=f"P4_{c}")
                nc.vector.tensor_tensor(v3(P3[:]), ayi_b, v3(rr[:]),
                                        op=OP.mult)
                nc.vector.tensor_tensor(v3(P4[:]), ayr_b, v3(ri[:]),
                                        op=OP.mult)
                hiv = v3(hc[c][:, KYZ:2 * KYZ], b=N)
                # +ky: hi = P4 + P3 ; -ky: hi = P4 - P3
                nc.vector.tensor_tensor(hiv[:, :, 0:33], v3(P4[:]),
                                        v3(P3[:]), op=OP.add)
                nc.vector.tensor_tensor(hiv[:, :, 33:65],
                                        v3(P4[:])[:, :, 1:33],
                                        v3(P3[:])[:, :, 1:33],
                                        op=OP.subtract)
            hr = [hc[c][:, 0:KYZ] for c in range(2)]
            hi = [hc[c][:, KYZ:2 * KYZ] for c in range(2)]

            with tc.tile_pool(name="psM", bufs=4, space="PSUM") as psM:
                # ---- phase F: spread + spectral multiply ----
                # hr pass over 4 chunks overlaps DVE building hi; psum ring
                # (4 x 2 banks) shared with the gather phase.
                Vc = gp.tile([N, 2 * KYZ], f16, tag="Vc")
                Vr, Vi = Vc[:, 0:KYZ], Vc[:, KYZ:2 * KYZ]
                pf = []

                def fmm_hr(k):
                    t = psM.tile([128, 1024], f32, tag="pq", name=f"pf{k}")
                    pf.append(t)
                    psr, psi = t[0:N, 0:CH], t[0:N, 512:512 + CH]
                    ch = slice(k * CH, (k + 1) * CH)
                    for c in range(2):
                        a = aT[c]
                        st = (c == 0)
                        nc.tensor.matmul(psr, a[:, 0:65], hr[c][:, ch],
                                         start=st, stop=False)
                        nc.tensor.matmul(psi, a[:, 65:130], hr[c][:, ch],
                                         start=st, stop=False)

                def fmm_hi(k):
                    t = pf[k]
                    psr, psi = t[0:N, 0:CH], t[0:N, 512:512 + CH]
                    ch = slice(k * CH, (k + 1) * CH)
                    for c in range(2):
                        a = aT[c]
                        sp_ = (c == 1)
                        nc.tensor.matmul(psr, a[:, 326:391], hi[c][:, ch],
                                         start=False, stop=sp_)
                        nc.tensor.matmul(psi, a[:, 0:65], hi[c][:, ch],
                                         start=False, stop=sp_)
                    # one DVE op: [Vr|Vi]_ch = [psr|psi] * W_ch
                    nc.vector.tensor_tensor(
                        v3(Vc[:], b=KYZ)[:, :, ch],
                        v3(t[0:N, :], b=512)[:, :, 0:CH],
                        Wt[:, ch].unsqueeze(1).broadcast_to([N, 2, CH]),
                        op=OP.mult)

                for k in range(4):
                    fmm_hr(k)
                for k in range(4):
                    fmm_hi(k)
                fmm_hr(4)
                fmm_hi(4)

                # ---- phase G + H: gather + fused multiply-reduce ----
                accT = []
                scr = []
                for c in range(2):
                    accT.append(sp.tile([128, NCHK], f32, tag=f"accT{c}",
                                        name=f"accT{c}"))
                    scr.append(sp.tile([128, 1024], f32, tag=f"scr{c}",
                                       name=f"scr{c}"))
                for k in range(NCHK):
                    ch = slice(k * CH, (k + 1) * CH)
                    for c in range(2):
                        axr_c = ax[:, c * 128:(c + 1) * 128]
                        axi_c = ax[:, 256 + c * 128:384 + c * 128]
                        naxi_c = ax[:, 512 + c * 128:640 + c * 128]
                        pg = psM.tile([128, 1024], f32, tag="pq",
                                      name=f"pg{c}_{k}")
                        pr, pi = pg[:, 0:CH], pg[:, 512:512 + CH]
                        nc.tensor.matmul(pr, axr_c, Vr[:, ch],
                                         start=True, stop=False)
                        nc.tensor.matmul(pr, axi_c, Vi[:, ch],
                                         start=False, stop=True)
                        nc.tensor.matmul(pi, axr_c, Vi[:, ch],
                                         start=True, stop=False)
                        nc.tensor.matmul(pi, naxi_c, Vr[:, ch],
                                         start=False, stop=True)
                        # one DVE op: accT[:,k] = sum(pr*hr + pi*hi)
                        nc.vector.scalar_tensor_tensor(
                            v3(scr[c][:], b=512)[:, :, 0:CH],
                            v3(pg[:], b=512)[:, :, 0:CH], 1.0,
                            v3(hc[c][:], b=KYZ)[:, :, ch],
                            op0=OP.mult, op1=OP.mult,
                            accum_out=accT[c][:, k:k + 1])
                for c in range(2):
                    fmm_c = sp.tile([128, 1], f32, tag=f"fmm{c}",
                                    name=f"fmm_{c}")
                    nc.vector.reduce_sum(fmm_c[:], accT[c][:], axis=AX.X)
                    nc.sync.dma_start(dout[c * 128:(c + 1) * 128, :],
                                      fmm_c[:])

    nc.compile()
    return nc


def _get_nc():
    if "nc" not in _CACHE:
        _CACHE["nc"] = _trace_kernel()
    return _CACHE["nc"]


def kernel(points, multRe0, multIm0, multRe1, multIm1):
    from concourse.bass_utils import run_bass_kernel_spmd

    points = np.asarray(points)
    multRe0 = np.asarray(multRe0)
    multRe1 = np.asarray(multRe1)
    multIm0 = np.asarray(multIm0)
    multIm1 = np.asarray(multIm1)

    Wfull = multRe0[0]
    ok = (np.all(multIm0 == 0) and np.all(multIm1 == 0)
          and np.array_equal(multRe0, multRe1)
          and np.array_equal(Wfull, Wfull[::-1, ::-1, ::-1]))
    if not ok:
        raise NotImplementedError("kernel specialized to symmetric real "
                                  "multipliers with equal channels")

    consts = _host_consts()
    Wk = _make_w(Wfull)

    ptsbx = np.zeros((B, N, 771), np.float32)
    for b in range(B):
        ptsbx[b, :, 0:768] = points[b].T.reshape(1, 768)
        ptsbx[b, :, 768:771] = consts["xb"]

    in_maps = []
    for b in range(B):
        in_maps.append({"ptsbx": ptsbx[b], "FF": consts["FF"], "W": Wk})

    nc = _get_nc()
    res = run_bass_kernel_spmd(nc, in_maps, core_ids=list(range(B)),
                               **_CACHE.get("run_kwargs", {}))
    _CACHE["last_result"] = res
    out = np.zeros((B, P, NCH), np.float32)
    for b in range(B):
        f = res.results[b]["fmm"][:, 0]
        out[b, :, 0] = f
        out[b, :, 1] = f
    return out


# revision 15
# speedup vs baseline: 1.1521x; 1.0179x over previous
"""NUFFT multi-channel 3D layer on 8 Trainium2 NeuronCores.

Strategy: data-parallel over batch (8 batches -> 8 cores). Per core the whole
pipeline runs in the Fourier domain:
 - gaussians computed grid-major via one fused Derivative_Erf activation per
   periodic image (exp(-u^2) table), no transposes anywhere;
 - 1D DFT factor planes in both orientations obtained directly by matmuls
   (forward: F as stationary; transposed: g as stationary);
 - Khatri-Rao product h = az (x) ay in kz-major layout; az replicated along
   ky by the scalar engine so the DVE products hit the packed-fp16 2x mode;
 - spread (particles->spectrum) and gather (spectrum->particles) as dense
   fp16 matmul streams, split into an hr pass and an hi pass so the tensor
   engine overlaps the DVE building hi; spectral multiply folded into fp16 W
   (hermitian weights on host, 1/N and sqrt(pi)/2 normalizations folded into
   the DFT matrices);
 - spread/gather PSUM pairs packed [re | im] into one 2-bank tile so the
   spectral multiply and the final contraction are one DVE op per chunk
   (scalar_tensor_tensor with accum_out for the fused multiply-reduce);
 - one shared 4-deep PSUM ring (8 banks) pipelines both matmul phases.
Hermitian symmetry halves the kz axis (33 of 65 planes).
"""
import sys
import numpy as np

sys.path.insert(0, "/opt/trn_rl_repo")

N = 65
NH = 33
P = 256
B = 8
L = 2.0 * np.pi
TAU = float(np.float32(12.0 * (np.float32(L) / (2.0 * np.pi * N)) ** 2))
NCH = 2
KYZ = N * NH          # 2145
CH = 429              # free chunk (5 chunks)
NCHK = 5

_CACHE = {}


def _host_consts():
    j = np.arange(N, dtype=np.float64)
    m = np.arange(N, dtype=np.float64) - 32.0
    Lf = float(np.float32(L))
    ph = -2.0 * np.pi * np.outer(m, j) / N           # [k, j]
    # per-axis deconv; 1/N (fft normalization split) and sqrt(pi)/2
    # (Derivative_Erf = 2/sqrt(pi) exp(-u^2)) folded in.
    dec = (np.sqrt(np.pi / TAU) * np.exp(m * m * TAU)
           * (np.sqrt(np.pi) / 2.0) / N)
    Fr = np.cos(ph) * dec[:, None]                   # [k, j]
    Fi = np.sin(ph) * dec[:, None]
    # FF: Fxr | Fxi | Fzr | Fzi  => [65, 196] fp16
    FF = np.ascontiguousarray(
        np.concatenate([Fr.T, Fi.T, Fr.T[:, 32:], Fi.T[:, 32:]], axis=1),
        np.float16)
    xg = np.linspace(0.0, Lf, N + 1)[:-1].astype(np.float64)
    s2t = 1.0 / (2.0 * np.sqrt(TAU))
    shifts = np.array([0.0, Lf, -Lf])
    xb = (-(xg[:, None] + shifts[None, :]) * s2t)    # [65, 3]
    return dict(FF=FF, xb=xb)


def _make_w(Wfull):
    # device layout: [kx, (kz-half, ky)] with ky order 0:33=+ky, 33:65=-ky
    kyperm = list(range(32, 65)) + list(range(31, -1, -1))
    w = np.ones(NH); w[1:] = 2.0
    Wk = np.asarray(Wfull, np.float64)[:, kyperm, 32:] * w[None, None, :]
    Wk = Wk.transpose(0, 2, 1).reshape(N, KYZ)
    return np.ascontiguousarray(Wk.astype(np.float16))


def _trace_kernel():
    import concourse.bass as bass
    import concourse.bacc as bacc
    import concourse.tile as tile
    from concourse import mybir

    dt = mybir.dt
    f32 = dt.float32
    f16 = dt.float16
    AF = mybir.ActivationFunctionType
    OP = mybir.AluOpType
    AX = mybir.AxisListType

    nc = bacc.Bacc("TRN2", target_bir_lowering=False, debug=False)

    din = {}
    for name, shape, ddt in [
            ("ptsbx", (N, 771), f32),      # pts broadcast 768 | xb 3
            ("FF", (N, 196), f16),         # Fxr | Fxi | Fzr | Fzi
            ("W", (N, KYZ), f16)]:
        din[name] = nc.dram_tensor(name, list(shape), ddt,
                                   kind="ExternalInput").ap()
    dout = nc.dram_tensor("fmm", [P, 1], f32, kind="ExternalOutput").ap()

    s2t = float(1.0 / (2.0 * np.sqrt(TAU)))

    def v3(ap, b=33):
        return ap.rearrange("p (a b) -> p a b", b=b)

    with tile.TileContext(nc) as tc:
        with (
            tc.tile_pool(name="const", bufs=1) as cp,
            tc.tile_pool(name="glob", bufs=1) as gp,
            tc.tile_pool(name="eph", bufs=2) as ep,
            tc.tile_pool(name="hpl", bufs=2) as hp,
            tc.tile_pool(name="scr", bufs=1) as sp,
        ):
            # ---- constants (+ activation-table preload during DMA) ----
            ptsbx = cp.tile([N, 771], f32, tag="ptsbx")
            nc.sync.dma_start(ptsbx[:, 0:384], din["ptsbx"][:, 0:384])
            nc.scalar.dma_start(ptsbx[:, 384:771], din["ptsbx"][:, 384:771])
            dmy = sp.tile([128, 1], f32, tag="dmy")
            nc.vector.memset(dmy[:], 0.0)
            dmo = sp.tile([128, 1], f16, tag="dmo")
            nc.scalar.activation(dmo[:], dmy[:], AF.Derivative_Erf)
            nc.scalar.activation(dmo[:], dmy[:], AF.Copy, scale=-1.0)
            FF = cp.tile([N, 196], f16, tag="FF")
            nc.sync.dma_start(FF[:], din["FF"][:])
            Wt = cp.tile([N, KYZ], f16, tag="W")
            nc.sync.dma_start(Wt[:], din["W"][:])
            ptsb = ptsbx[:, 0:768]
            xb = ptsbx[:, 768:771]
            Fx, Fz = FF[:, 0:130], FF[:, 130:196]

            # ---- phase A: periodic gaussians, grid-major [x, (a p)] ----
            e3 = []
            for s in range(3):
                e = gp.tile([N, 768], f16, tag=f"e{s}")
                nc.scalar.activation(e[:], ptsb, AF.Derivative_Erf,
                                     bias=xb[:, s:s + 1], scale=s2t)
                e3.append(e)
            g = gp.tile([N, 768], f16, tag="g")
            nc.vector.tensor_tensor(g[:], e3[0][:], e3[1][:], op=OP.add)
            nc.vector.tensor_tensor(g[:], g[:], e3[2][:], op=OP.add)
            gx, gy, gz = g[:, 0:256], g[:, 256:512], g[:, 512:768]

            aT = []
            rep = []
            pTs = []
            with tc.tile_pool(name="psC", bufs=1, space="PSUM") as psC:
                # ---- phase C: transposed DFT planes per particle chunk ----
                # aT[c] cols: axTr 0:65 | axTi 65:130 | ayTr 130:195 |
                #             ayTi 195:260 | azTr 260:293 | azTi 293:326 |
                #             naxTi 326:391
                for c in range(2):
                    cs = slice(c * 128, (c + 1) * 128)
                    pT = psC.tile([128, 326], f32, tag="pT", name=f"pT{c}")
                    nc.tensor.matmul(pT[:, 0:130], gx[:, cs], Fx,
                                     start=True, stop=True)
                    nc.tensor.matmul(pT[:, 130:260], gy[:, cs], Fx,
                                     start=True, stop=True)
                    nc.tensor.matmul(pT[:, 260:326], gz[:, cs], Fz,
                                     start=True, stop=True)
                    t = gp.tile([128, 391], f16, tag=f"aT{c}")
                    nc.scalar.copy(t[:, 0:326], pT[:])
                    aT.append(t)
                    pTs.append(pT)
                    # az replicated along ky (scalar) so the DVE Khatri-Rao
                    # products run in the packed-fp16 2x mode
                    rr = ep.tile([128, 1089], f16, tag="rr", name=f"rr{c}")
                    nc.scalar.copy(
                        v3(rr[:]),
                        t[:, 260:293].unsqueeze(2).broadcast_to([128, 33, 33]))
                    ri = ep.tile([128, 1089], f16, tag="ri", name=f"ri{c}")
                    nc.scalar.copy(
                        v3(ri[:]),
                        t[:, 293:326].unsqueeze(2).broadcast_to([128, 33, 33]))
                    rep.append((rr, ri))
                # ---- forward ax planes [kx, p] ----
                psax = psC.tile([N, 512], f32, tag="psax", name="psax")
                nc.tensor.matmul(psax[:, 0:256], Fx[:, 0:65], gx,
                                 start=True, stop=True)
                nc.tensor.matmul(psax[:, 256:512], Fx[:, 65:130], gx,
                                 start=True, stop=True)
                # negated axTi, needed only by the spread's hi pass
                for c in range(2):
                    nc.scalar.activation(aT[c][:, 326:391],
                                         pTs[c][:, 65:130],
                                         AF.Copy, scale=-1.0)
                # ax cols: axr 0:256 | axi 256:512 | naxi 512:768
                ax = gp.tile([N, 768], f16, tag="ax")
                nc.scalar.copy(ax[:, 0:512], psax[:])
                nc.scalar.activation(ax[:, 512:768], psax[:, 256:512],
                                     AF.Copy, scale=-1.0)

            # ---- phase E: h = az (x) ay, kz-major [p, (kz, ky65)] ----
            # DVE order: P1,P2 + hr combines per c first (unblocks the
            # spread's hr matmul pass), then P3,P4 + hi combines.
            hc, PP = [], []
            for c in range(2):
                t = aT[c]
                rr, ri = rep[c]
                ayr_b = t[:, 162:195].unsqueeze(1).broadcast_to([128, 33, 33])
                ayi_b = t[:, 227:260].unsqueeze(1).broadcast_to([128, 33, 33])
                P1 = ep.tile([128, 1089], f16, tag="P1", name=f"P1_{c}")
                P2 = ep.tile([128, 1089], f16, tag="P2", name=f"P2_{c}")
                nc.vector.tensor_tensor(v3(P1[:]), ayr_b, v3(rr[:]),
                                        op=OP.mult)
                nc.vector.tensor_tensor(v3(P2[:]), ayi_b, v3(ri[:]),
                                        op=OP.mult)
                hcat = hp.tile([128, 2 * KYZ], f16, tag="hcat",
                               name=f"hcat{c}")
                hrv = v3(hcat[:, 0:KYZ], b=N)     # [p, kz, ky]
                # +ky block: hr = P1 - P2 ; -ky block (ky 1:33): P1 + P2
                nc.vector.tensor_tensor(hrv[:, :, 0:33], v3(P1[:]),
                                        v3(P2[:]), op=OP.subtract)
                nc.vector.tensor_tensor(hrv[:, :, 33:65],
                                        v3(P1[:])[:, :, 1:33],
                                        v3(P2[:])[:, :, 1:33], op=OP.add)
                hc.append(hcat)
                PP.append((ayr_b, ayi_b))
            for c in range(2):
                ayr_b, ayi_b = PP[c]
                rr, ri = rep[c]
                P3 = ep.tile([128, 1089], f16, tag="P3", name=f"P3_{c}")
                P4 = ep.tile([128, 1089], f16, tag="P4", name=f"P4_{c}")
                nc.vector.tensor_tensor(v3(P3[:]), ayi_b, v3(rr[:]),
                                        op=OP.mult)
                nc.vector.tensor_tensor(v3(P4[:]), ayr_b, v3(ri[:]),
                                        op=OP.mult)
                hiv = v3(hc[c][:, KYZ:2 * KYZ], b=N)
                # +ky: hi = P4 + P3 ; -ky: hi = P4 - P3
                nc.vector.tensor_tensor(hiv[:, :, 0:33], v3(P4[:]),
                                        v3(P3[:]), op=OP.add)
                nc.vector.tensor_tensor(hiv[:, :, 33:65],
                                        v3(P4[:])[:, :, 1:33],
                                        v3(P3[:])[:, :, 1:33],
                                        op=OP.subtract)
            hr = [hc[c][:, 0:KYZ] for c in range(2)]
            hi = [hc[c][:, KYZ:2 * KYZ] for c in range(2)]

            with tc.tile_pool(name="psM", bufs=4, space="PSUM") as psM:
                # ---- phase F: spread + spectral multiply ----
                # hr pass over 4 chunks overlaps DVE building hi; psum ring
                # (4 x 2 banks) shared with the gather phase.
                Vc = gp.tile([N, 2 * KYZ], f16, tag="Vc")
                Vr, Vi = Vc[:, 0:KYZ], Vc[:, KYZ:2 * KYZ]
                pf = []

                def fmm_hr(k):
                    t = psM.tile([128, 1024], f32, tag="pq", name=f"pf{k}")
                    pf.append(t)
                    psr, psi = t[0:N, 0:CH], t[0:N, 512:512 + CH]
                    ch = slice(k * CH, (k + 1) * CH)
                    for c in range(2):
                        a = aT[c]
                        st = (c == 0)
                        nc.tensor.matmul(psr, a[:, 0:65], hr[c][:, ch],
                                         start=st, stop=False)
                        nc.tensor.matmul(psi, a[:, 65:130], hr[c][:, ch],
                                         start=st, stop=False)

                def fmm_hi(k):
                    t = pf[k]
                    psr, psi = t[0:N, 0:CH], t[0:N, 512:512 + CH]
                    ch = slice(k * CH, (k + 1) * CH)
                    for c in range(2):
                        a = aT[c]
                        sp_ = (c == 1)
                        nc.tensor.matmul(psr, a[:, 326:391], hi[c][:, ch],
                                         start=False, stop=sp_)
                        nc.tensor.matmul(psi, a[:, 0:65], hi[c][:, ch],
                                         start=False, stop=sp_)
                    # one DVE op: [Vr|Vi]_ch = [psr|psi] * W_ch
                    nc.vector.tensor_tensor(
                        v3(Vc[:], b=KYZ)[:, :, ch],
                        v3(t[0:N, :], b=512)[:, :, 0:CH],
                        Wt[:, ch].unsqueeze(1).broadcast_to([N, 2, CH]),
                        op=OP.mult)

                for k in range(4):
                    fmm_hr(k)
                for k in range(4):
                    fmm_hi(k)
                fmm_hr(4)
                fmm_hi(4)

                # ---- phase G + H: gather + fused multiply-reduce ----
                accT = []
                scr = []
                for c in range(2):
                    accT.append(sp.tile([128, NCHK], f32, tag=f"accT{c}",
                                        name=f"accT{c}"))
                    scr.append(sp.tile([128, 1024], f32, tag=f"scr{c}",
                                       name=f"scr{c}"))
                for k in range(NCHK):
                    ch = slice(k * CH, (k + 1) * CH)
                    for c in range(2):
                        axr_c = ax[:, c * 128:(c + 1) * 128]
                        axi_c = ax[:, 256 + c * 128:384 + c * 128]
                        naxi_c = ax[:, 512 + c * 128:640 + c * 128]
                        pg = psM.tile([128, 1024], f32, tag="pq",
                                      name=f"pg{c}_{k}")
                        pr, pi = pg[:, 0:CH], pg[:, 512:512 + CH]
                        nc.tensor.matmul(pr, axr_c, Vr[:, ch],
                                         start=True, stop=False)
                        nc.tensor.matmul(pr, axi_c, Vi[:, ch],
                                         start=False, stop=True)
                        nc.tensor.matmul(pi, axr_c, Vi[:, ch],
                                         start=True, stop=False)
                        nc.tensor.matmul(pi, naxi_c, Vr[:, ch],
                                         start=False, stop=True)
                        # one DVE op: accT[:,k] = sum(pr*hr + pi*hi)
                        nc.vector.scalar_tensor_tensor(
                            v3(scr[c][:], b=512)[:, :, 0:CH],
                            v3(pg[:], b=512)[:, :, 0:CH], 1.0,
                            v3(hc[c][:], b=KYZ)[:, :, ch],
                            op0=OP.mult, op1=OP.mult,
                            accum_out=accT[c][:, k:k + 1])
                for c in range(2):
                    fmm_c = sp.tile([128, 1], f32, tag=f"fmm{c}",
                                    name=f"fmm_{c}")
                    nc.vector.reduce_sum(fmm_c[:], accT[c][:], axis=AX.X)
                    nc.sync.dma_start(dout[c * 128:(c + 1) * 128, :],
                                      fmm_c[:])

    nc.compile()
    return nc


def _get_nc():
    if "nc" not in _CACHE:
        _CACHE["nc"] = _trace_kernel()
    return _CACHE["nc"]


def kernel(points, multRe0, multIm0, multRe1, multIm1):
    from concourse.bass_utils import run_bass_kernel_spmd

    points = np.asarray(points)
    multRe0 = np.asarray(multRe0)
    multRe1 = np.asarray(multRe1)
    multIm0 = np.asarray(multIm0)
    multIm1 = np.asarray(multIm1)

    Wfull = multRe0[0]
    ok = (np.all(multIm0 == 0) and np.all(multIm1 == 0)
          and np.array_equal(multRe0, multRe1)
          and np.array_equal(Wfull, Wfull[::-1, ::-1, ::-1]))
    if not ok:
        raise NotImplementedError("kernel specialized to symmetric real "
                                  "multipliers with equal channels")

    consts = _host_consts()
    Wk = _make_w(Wfull)

    ptsbx = np.zeros((B, N, 771), np.float32)
    for b in range(B):
        ptsbx[b, :, 0:768] = points[b].T.reshape(1, 768)
        ptsbx[b, :, 768:771] = consts["xb"]

    in_maps = []
    for b in range(B):
        in_maps.append({"ptsbx": ptsbx[b], "FF": consts["FF"], "W": Wk})

    nc = _get_nc()
    res = run_bass_kernel_spmd(nc, in_maps, core_ids=list(range(B)),
                               **_CACHE.get("run_kwargs", {}))
    _CACHE["last_result"] = res
    out = np.zeros((B, P, NCH), np.float32)
    for b in range(B):
        f = res.results[b]["fmm"][:, 0]
        out[b, :, 0] = f
        out[b, :, 1] = f
    return out
